# revision 1
# baseline (speedup 1.0000x reference)
"""Trainium2 Bass kernel for nn_EyeRobotAgent block-sparse ("eye") attention.

Shapes: q,k,v [2, 12, 3456, 32] fp32.  S = 16 time-blocks x 216 feats.
Mask structure (per query block t):
  - all 216 keys of block t are candidates (minus img->img),
  - of each past block t-7..t-1, only 19 keys (m in {0..3, 5..19}) are
    visible (proprio m==4 and img m>=20 keys are never visible in the past),
  - joint queries (m in [4,20)) cannot see past joint keys,
  - img queries (m >= 20) cannot see img keys at all.

Strategy (data-parallel: 24 (b,h) pairs over 8 cores, 3 each):
  Pack per block t a compact key set [216 same | 133 past | 35 pad] = 384.
  The 2-D mask folds into the QK matmul via 3 extra contraction rows
  (rank-1 decomposition of the mask predicates); invalid/pad columns get a
  large negative bias so exp() underflows to 0.  Scores are computed
  transposed [kv, q] so probs can be consumed directly by the PV matmul,
  with a ones-column appended to V producing softmax denominators.
  exp() has no max-subtraction (scores are O(6), fp32-safe).
"""
import numpy as np

import concourse.bass as bass
import concourse.mybir as mybir
import concourse.tile as tile
from concourse import bacc
from concourse.bass_utils import run_bass_kernel_spmd
from concourse.masks import make_identity
from concourse.tile_rust import add_dep_helper

B, H, S, D = 2, 12, 3456, 32
F = 216            # feats_per_t
W = 8              # window_len
T = S // F         # 16 blocks
IMG_START = 20     # F - img_feat_size
JOINT_START = 4    # IMG_START - act_size
PAST_SEL = np.array([0, 1, 2, 3] + list(range(5, 20)))   # 19 per past block
NPAST = 19 * (W - 1)     # 133
KV = 384                 # packed kv per block (216 + 133 + pad)
KAUG = D + 3             # 35 contraction rows (32 d + 3 mask-bias rows)
VA = D + 1               # 33 = v columns + ones column
NEG = np.float32(-30000.0)
SCALE = float(1.0 / np.sqrt(np.float32(D)))
N_CORES = 8
BH_PER_CORE = (B * H) // N_CORES      # 3
NPAIR = T // 2                        # 8 block-pairs per (b,h)

F32 = mybir.dt.float32
BF16 = mybir.dt.float16      # half precision: matmul rate 1 cyc/row, 10-bit mantissa
NP_BF16 = np.float16


# ---------------------------------------------------------------- host packing

def _pack_all(q, k, v):
    """q,k,v: [B,H,S,D] fp32 ->
       qt  [24, KAUG, S]     (augmented Q^T)
       kpt [24, T, KAUG, KV] (augmented packed K^T per block)
       vp  [24, T, KV, VA]   (packed V + ones column per block)"""
    nbh = B * H
    qf = q.reshape(nbh, S, D)
    kf = k.reshape(nbh, S, D)
    vf = v.reshape(nbh, S, D)

    m = np.arange(F)
    is_img_m = (m >= IMG_START).astype(np.float32)
    is_joint_m = ((m >= JOINT_START) & (m < IMG_START)).astype(np.float32)
    qm = np.arange(S) % F

    qt = np.zeros((nbh, KAUG, S), np.float32)
    qt[:, :D] = qf.transpose(0, 2, 1)
    qt[:, 32] = (qm >= IMG_START)
    qt[:, 33] = (qm >= JOINT_START) & (qm < IMG_START)
    qt[:, 34] = 1.0

    kpt = np.zeros((nbh, T, KAUG, KV), np.float32)
    vp = np.zeros((nbh, T, KV, VA), np.float32)
    joint_past_bias = np.tile(NEG * is_joint_m[PAST_SEL], W - 1)  # [133]
    for t in range(T):
        blk = slice(F * t, F * (t + 1))
        kpt[:, t, :D, :F] = kf[:, blk].transpose(0, 2, 1)
        kpt[:, t, 32, :F] = NEG * is_img_m
        vp[:, t, :F, :D] = vf[:, blk]
        vp[:, t, :F, 32] = 1.0
        # past blocks t-7 .. t-1, 19 keys each
        taus = np.arange(t - 7, t)
        rows = (F * taus[:, None] + PAST_SEL[None, :]).reshape(-1)   # [133]
        valid = np.repeat(taus >= 0, 19)                             # [133]
        safe_rows = np.where(valid, rows, 0)
        pc = slice(F, F + NPAST)
        kpt[:, t, :D, pc] = np.where(
            valid[None, None, :], kf[:, safe_rows].transpose(0, 2, 1), 0.0)
        kpt[:, t, 33, pc] = joint_past_bias
        kpt[:, t, 34, pc] = np.where(valid, 0.0, NEG)
        vp[:, t, pc, :D] = np.where(
            valid[None, :, None], vf[:, safe_rows], 0.0)
        vp[:, t, pc, 32] = valid
        kpt[:, t, 34, F + NPAST:] = NEG        # pad columns
    # bulk per-bh DMA layouts:
    #   kpt2[bh, r, t, c]        = kpt[bh, t, r, c]
    #   vp2[bh, p, pair, c, tb*VA+n] = vp[bh, 2*pair+tb, 128*c+p, n]
    kpt2 = np.ascontiguousarray(kpt.transpose(0, 2, 1, 3))
    vp2 = vp.reshape(nbh, T // 2, 2, 3, 128, VA).transpose(0, 4, 1, 3, 2, 5)
    vp2 = np.ascontiguousarray(vp2.reshape(nbh, 128, T // 2, 3, 2 * VA))
    return (qt.astype(NP_BF16), kpt2.astype(NP_BF16), vp2.astype(NP_BF16))


# ---------------------------------------------------------------- bass kernel

def build_nc(n_bh=BH_PER_CORE, n_pairs=NPAIR):
    nc = bacc.Bacc(None, target_bir_lowering=False, debug=False)
    qt_d = nc.declare_dram_parameter("qt", [BH_PER_CORE, KAUG, S], BF16, isOutput=False)
    kpt_d = nc.declare_dram_parameter("kpt", [BH_PER_CORE, KAUG, T, KV], BF16, isOutput=False)
    vp_d = nc.declare_dram_parameter("vp", [BH_PER_CORE, 128, T // 2, 3, 2 * VA], BF16, isOutput=False)
    out_d = nc.declare_dram_parameter("out", [BH_PER_CORE, S, D], F32, isOutput=True)

    def _strided2(ap, d1, d2):
        return bass.AP(tensor=ap.tensor, offset=ap.offset,
                       ap=[list(ap.ap[0]), list(d1), list(d2)])

    with tile.TileContext(nc) as tc:
        with (
            tc.tile_pool(name="singles", bufs=1) as singles,
            tc.tile_pool(name="qtp", bufs=3) as qtp,
            tc.tile_pool(name="kptp", bufs=3) as kptp,
            tc.tile_pool(name="vpp", bufs=3) as vpp,
            tc.tile_pool(name="probsp", bufs=3) as probsp,
            tc.tile_pool(name="pvsbp", bufs=3) as pvsbp,
            tc.tile_pool(name="recipsp", bufs=3) as recipsp,
            tc.tile_pool(name="outsbp", bufs=3) as outsbp,
            tc.tile_pool(name="scoresp", bufs=2, space="PSUM") as scoresp,
            tc.tile_pool(name="pvp", bufs=1, space="PSUM") as pvp,
        ):
            ident = singles.tile([128, 128], F32)
            make_identity(nc, ident[:])

            for i in range(n_bh):
                qt_sb = qtp.tile([KAUG, S], BF16)
                kpt_sb = kptp.tile([KAUG, T, KV], BF16)
                vp_sb = vpp.tile([128, T // 2, 3, 2 * VA], BF16)
                for hf in range(2):
                    hs, ts_, ps_ = S // 2 * hf, T // 2 * hf, NPAIR // 2 * hf
                    nc.sync.dma_start(out=qt_sb[:, hs:hs + S // 2],
                                      in_=qt_d[i, :, hs:hs + S // 2])
                    nc.sync.dma_start(out=kpt_sb[:, ts_:ts_ + T // 2, :],
                                      in_=kpt_d[i, :, ts_:ts_ + T // 2, :])
                    nc.sync.dma_start(
                        out=vp_sb[:, ps_:ps_ + NPAIR // 2, :, :],
                        in_=vp_d[i, :, ps_:ps_ + NPAIR // 2, :, :])
                outst = outsbp.tile([128, NPAIR * 128], F32)


                for p in range(n_pairs):
                    t0 = 2 * p

                    # ---- QK^T (transposed scores [kv, q]), mask via bias rows
                    scores = scoresp.tile([128, 1536], F32)   # 3 psum banks
                    for c in range(3):
                        first = None
                        for tb in range(2):
                            mm = nc.tensor.matmul(
                                scores[:, 512 * c + 216 * tb:512 * c + 216 * tb + 216],
                                lhsT=kpt_sb[:, t0 + tb, 128 * c:128 * c + 128],
                                rhs=qt_sb[:, 216 * (t0 + tb):216 * (t0 + tb) + 216],
                                start=(tb == 0), stop=(tb == 1))
                            if tb == 0:
                                first = mm
                            else:
                                add_dep_helper(mm.ins, first.ins, sync=False,
                                               reason="qk same-bank group order")

                    # ---- probs = exp(scale * scores), one ACT op
                    probs = probsp.tile([128, 1296], BF16)
                    sc_v = scores[:].rearrange("p (c x) -> p c x", c=3)[:, :, 0:432]
                    pr_v = probs[:].rearrange("p (c x) -> p c x", c=3)
                    nc.scalar.activation(pr_v, sc_v,
                                         mybir.ActivationFunctionType.Exp,
                                         scale=SCALE)

                    # ---- PV: out_T[va, q]; one psum bank per block
                    # (separate banks avoid interleaved-group pending-zero
                    # hazards and cross-engine bank overlap).
                    pv = pvp.tile([128, 1024], F32)
                    for tb in range(2):
                        for c in range(3):
                            nc.tensor.matmul(
                                pv[0:VA, 512 * tb:512 * tb + 216],
                                lhsT=vp_sb[:, p, c, VA * tb:VA * tb + VA],
                                rhs=probs[:, 432 * c + 216 * tb:432 * c + 216 * tb + 216],
                                start=(c == 0), stop=(c == 2))

                    # ---- evacuate both blocks to SBUF in one DVE op
                    pvsb = pvsbp.tile([VA, 2, 216], F32)
                    cp1 = nc.vector.tensor_copy(
                        pvsb[:],
                        _strided2(pv[0:VA, 0:1], (512, 2), (1, 216)))

                    # ---- PE-transpose each q-slice into the scores tile's
                    # spare columns (exp already consumed those banks; this
                    # frees the pv tile for the next pair right after cp1).
                    # qs=0 slots (128-wide) -> bank0 col 432+33*tb;
                    # qs=1 slots ( 88-wide) -> bank1 col 944+33*tb.
                    prevs = [None, None]
                    for tb in range(2):
                        for qs in range(2):
                            w = 128 if qs == 0 else 88
                            col = (432 if qs == 0 else 944) + 33 * tb
                            mm = nc.tensor.matmul(
                                scores[0:w, col:col + VA],
                                lhsT=pvsb[:, tb, 128 * qs:128 * qs + w],
                                rhs=ident[0:VA, 0:VA],
                                is_transpose=True,
                                start=(tb == 0), stop=(tb == 1))
                            if prevs[qs] is not None:
                                add_dep_helper(mm.ins, prevs[qs].ins,
                                               sync=False,
                                               reason="tr bank group order")
                            prevs[qs] = mm

                    # ---- normalize: out = num * (1/den), split by q-width
                    # (all reader->writer hazards here are RAW-tracked: each
                    # recip/mul reads both of its bank's transpose outputs)
                    recips = recipsp.tile([128, 4], F32)
                    nc.vector.reciprocal(recips[0:128, 0:3:2],
                                         scores[0:128, 464:498:33])
                    nc.vector.reciprocal(recips[0:88, 1:4:2],
                                         scores[0:88, 976:1010:33])
                    _strided = _strided2

                    for qs, w in ((0, 128), (1, 88)):
                        col = 432 if qs == 0 else 944
                        num_v = _strided(scores[0:w, col:col + 1],
                                         (33, 2), (1, 32))
                        rec_bcast = _strided(recips[0:w, qs:qs + 1],
                                             (2, 2), (0, 32))
                        out_v = _strided(
                            outst[0:w, 128 * p + 32 * qs:128 * p + 32 * qs + 1],
                            (64, 2), (1, 32))
                        nc.vector.tensor_mul(out_v, num_v, rec_bcast)

                    # ---- store half-(b,h) after pairs 0-3 / 4-7 complete
                    if p % (NPAIR // 2) == NPAIR // 2 - 1:
                        hf = p // (NPAIR // 2)
                        po = NPAIR // 2 * hf
                        for qs, w in ((0, 128), (1, 88)):
                            dst = bass.AP(
                                tensor=out_d,
                                offset=(i * S + 432 * po + 128 * qs) * D,
                                ap=[[D, w], [432 * D, NPAIR // 2],
                                    [216 * D, 2], [1, D]])
                            sap = outst[:]
                            sst = bass.AP(
                                tensor=sap.tensor,
                                offset=sap.offset + 128 * po + 32 * qs,
                                ap=[[sap.ap[0][0], w], [128, NPAIR // 2],
                                    [64, 2], [1, 32]])
                            nc.sync.dma_start(out=dst, in_=sst)
    nc.compile()
    return nc


_NC = None


def _get_nc():
    global _NC
    if _NC is None:
        _NC = build_nc()
    return _NC


# ---------------------------------------------------------------- entry point

def kernel(q, k, v, feats_per_t, window_len, act_size, img_feat_size):
    assert int(feats_per_t) == F and int(window_len) == W
    assert int(act_size) == 16 and int(img_feat_size) == 196
    q = np.asarray(q, np.float32)
    k = np.asarray(k, np.float32)
    v = np.asarray(v, np.float32)

    qt, kpt, vp = _pack_all(q, k, v)
    in_maps = []
    for core in range(N_CORES):
        s = slice(BH_PER_CORE * core, BH_PER_CORE * (core + 1))
        in_maps.append({"qt": np.ascontiguousarray(qt[s]),
                        "kpt": np.ascontiguousarray(kpt[s]),
                        "vp": np.ascontiguousarray(vp[s])})

    nc = _get_nc()
    res = run_bass_kernel_spmd(nc, in_maps, list(range(N_CORES)))
    out = np.empty((B * H, S, D), np.float32)
    for core in range(N_CORES):
        out[BH_PER_CORE * core:BH_PER_CORE * (core + 1)] = res.results[core]["out"]
    return out.reshape(B, H, S, D)

